# revision 67
# baseline (speedup 1.0000x reference)
import numpy as np
import concourse.bass as bass
import concourse.mybir as mybir
import concourse.tile as tile
from concourse import bacc
from concourse.bass_utils import run_bass_kernel_spmd

B, S, D, H, DH = 4, 2048, 768, 12, 64
HPC = 6          # heads per core
PAIRS = HPC // 2
THETA = 10000.0
N_CORES = 8
F32 = mybir.dt.float32
F32R = mybir.dt.float32r
F16 = mybir.dt.float16
VW = HPC * 65    # V block width: 6 heads x (64 + ones col)

# stream_shuffle swaps the 16-row even/odd halves within each 32-partition
# quadrant (RoPE rotate-half, see make_in_maps for the row layout)
SHUF_MASK = [(i + 16) % 32 for i in range(32)]

_NC = None


def interleave(main_units, extra_units):
    """Merge two unit lists, spreading extra_units evenly among main_units."""
    n, m = len(main_units), len(extra_units)
    if n == 0:
        return list(extra_units)
    res, j = [], 0
    for i, u in enumerate(main_units):
        res.append(u)
        while j < m and (j + 1) * n <= (i + 1) * m:
            res.append(extra_units[j])
            j += 1
    res.extend(extra_units[j:])
    return res


def build_nc(with_collective=True):
    nc = bacc.Bacc("TRN2", target_bir_lowering=False, debug=False,
                   num_devices=N_CORES)
    xT = nc.dram_tensor("xT", [D, S], F16, kind="ExternalInput")
    wqT = nc.dram_tensor("wqT", [D, 384], F16, kind="ExternalInput")
    wkT = nc.dram_tensor("wkT", [D, 384], F16, kind="ExternalInput")
    wv = nc.dram_tensor("wv", [D, VW], F16, kind="ExternalInput")
    wo = nc.dram_tensor("wo", [384, D], F16, kind="ExternalInput")
    cosd = nc.dram_tensor("cos", [128, S], F16, kind="ExternalInput")
    sind = nc.dram_tensor("sin", [128, S], F16, kind="ExternalInput")
    maskd = nc.dram_tensor("mask", [128, 128], F16, kind="ExternalInput")
    idend = nc.dram_tensor("iden", [128, 128], F16, kind="ExternalInput")
    out = nc.dram_tensor("out", [S, D], F32, kind="ExternalOutput")

    with tile.TileContext(nc) as tc:
        with tc.tile_pool(name="persist", bufs=1) as pp, \
             tc.tile_pool(name="dram", bufs=1, space="DRAM") as dpool, \
             tc.tile_pool(name="xp", bufs=2) as xp, \
             tc.tile_pool(name="ru", bufs=6) as ru, \
             tc.tile_pool(name="et", bufs=4) as etp, \
             tc.tile_pool(name="nrm", bufs=6) as nrm, \
             tc.tile_pool(name="cxq", bufs=3) as cxq, \
             tc.tile_pool(name="ot", bufs=3) as otp, \
             tc.tile_pool(name="pps", bufs=3, space="PSUM") as pps, \
             tc.tile_pool(name="pcx", bufs=2, space="PSUM") as pcx:
            sb_q = [pp.tile([128, S], F16, name=f"sb_q{i}") for i in range(PAIRS)]
            sb_k = [pp.tile([128, S], F16, name=f"sb_k{i}") for i in range(PAIRS)]
            sb_v = pp.tile([128, 96, 65], F16)
            sb_ctxT3 = pp.tile([128, PAIRS, S], F16)
            sb_ctxT = [sb_ctxT3[:, i, :] for i in range(PAIRS)]
            sb_wo = pp.tile([128, 3 * D], F16)
            sb_mask = pp.tile([128, 128], F16)
            sb_iden = pp.tile([128, 128], F16)
            sb_wq = pp.tile([128, 6 * 384], F16)
            sb_wk = pp.tile([128, 6 * 384], F16)
            sb_wv = pp.tile([128, 6 * VW], F16)
            sb_cos = pp.tile([128, S], F16)
            sb_sin = pp.tile([128, S], F16)
            bounce_in = dpool.tile([S, D], F32)
            bounce_out = dpool.tile([S, D], F32)
            out_dram = bounce_in if with_collective else out

            xcols = [None] * 4

            def load_x(tt):
                xcols[tt] = xp.tile([128, 6 * 512], F16, name="xcol")
                for ck in range(6):
                    nc.sync.dma_start(
                        xcols[tt][:, ck * 512:(ck + 1) * 512],
                        xT[ck * 128:(ck + 1) * 128,
                           tt * 512:(tt + 1) * 512])

            def load_mat(eng, dst, src, w):
                # one strided DMA: [nchunk*128, w] DRAM -> [128, nchunk, w]
                eng.dma_start(
                    dst[:].rearrange("p (a c) -> p a c", c=w),
                    src[:, 0:w].rearrange("(a p) c -> p a c", p=128))

            # startup: x0 per-chunk on alternating SP/Act HWDGE queues,
            # weights on SWDGE, in first-use order (x0+wq -> Q0, wk -> K0,
            # cos/sin -> rope, wv -> V)
            def load_wpair(dst, src, p):
                # one strided DMA for head-pair p's 128 weight columns
                nc.sync.dma_start(
                    dst[:].rearrange("r (a c) -> r a c", c=384)[:, :,
                                                              p * 128:
                                                              (p + 1) * 128],
                    src[:, p * 128:(p + 1) * 128].rearrange(
                        "(a r) c -> r a c", r=128))

            # startup feed order = first-use order: pair-0 Q/K weights and
            # cos/sin arrive before the later pairs
            xcols[0] = xp.tile([128, 6 * 512], F16, name="xcol")

            def load_x0(ck):
                nc.sync.dma_start(
                    xcols[0][:, ck * 512:(ck + 1) * 512],
                    xT[ck * 128:(ck + 1) * 128, 0:512])

            load_x0(0)
            load_x0(1)
            load_wpair(sb_wq, wqT, 0)
            load_wpair(sb_wk, wkT, 0)
            load_x0(2)
            load_x0(3)
            nc.sync.dma_start(sb_cos[:], cosd[:])
            nc.sync.dma_start(sb_sin[:], sind[:])
            load_x0(4)
            load_x0(5)
            load_wpair(sb_wq, wqT, 1)
            load_wpair(sb_wk, wkT, 1)
            nc.sync.dma_start(sb_mask[:], maskd[:])
            for ck in range(6):
                nc.sync.dma_start(sb_wv[:, ck * VW:(ck + 1) * VW],
                                  wv[ck * 128:(ck + 1) * 128, :])
            load_wpair(sb_wq, wqT, 2)
            load_wpair(sb_wk, wkT, 2)
            nc.sync.dma_start(sb_iden[:], idend[:])
            for ci in range(3):
                nc.sync.dma_start(sb_wo[:, ci * D:(ci + 1) * D],
                                  wo[ci * 128:(ci + 1) * 128, :])

            def proj_units(tt):
                """QKV projection + RoPE for token block tt as emit-units."""
                xcol = xcols[tt]
                csl = sb_cos[:, tt * 512:(tt + 1) * 512]
                ssl = sb_sin[:, tt * 512:(tt + 1) * 512]

                def qk_unit(wc, dst, p, wi):
                    pqt = pps.tile([128, 1024], F32, name="psc")
                    pc = pqt[:, 0:512]
                    for ck in range(6):
                        nc.tensor.matmul(
                            pc[:],
                            wc[:, ck * 384 + p * 128:ck * 384 + (p + 1) * 128],
                            xcol[:, ck * 512:(ck + 1) * 512],
                            start=(ck == 0), stop=(ck == 5))
                    dsl = dst[p][:, tt * 512:(tt + 1) * 512]
                    uh = ru.tile([128, 512], F16)
                    uhs = ru.tile([128, 512], F16)
                    with nc.allow_low_precision(reason="f16 q/k"):
                        nc.vector.tensor_mul(dsl, pc[:], csl)
                        nc.vector.tensor_mul(uh[:], pc[:], ssl)
                        nc.vector.stream_shuffle(uhs[:], uh[:], SHUF_MASK)
                        nc.vector.tensor_add(dsl, dsl, uhs[:])

                def v_unit(tj):
                    tb = tt * 4 + tj
                    pvq = pps.tile([128, 1024], F32, name="psc")
                    pvt = pvq[:, 0:512]
                    for ck in range(6):
                        nc.tensor.matmul(
                            pvt[:, 0:VW],
                            xcol[:, ck * 512 + tj * 128:
                                    ck * 512 + tj * 128 + 128],
                            sb_wv[:, ck * VW:(ck + 1) * VW],
                            start=(ck == 0), stop=(ck == 5))
                    with nc.allow_low_precision(reason="f16 V store"):
                        # late-block V copies on DVE: Act saturates with exp
                        # work in the final regions
                        dst_v = sb_v[:, tb * 6:(tb + 1) * 6, :]
                        src_v = pvt[:, 0:VW].rearrange("p (a b) -> p a b", b=65)
                        if tt < 3:
                            nc.scalar.copy(dst_v, src_v)
                        else:
                            nc.vector.tensor_copy(dst_v, src_v)
                    nc.gpsimd.memset(sb_v[:, tb * 6:(tb + 1) * 6, 64:65], 1.0)

                vu = [lambda tj=tj: v_unit(tj) for tj in range(4)]
                qp = [[lambda p=p, wi=wi, wc=wc, dst=dst: qk_unit(wc, dst, p, wi)
                       for wi, (wc, dst) in enumerate(((sb_wq, sb_q),
                                                       (sb_wk, sb_k)))]
                      for p in range(PAIRS)]
                # pair-major [Qp, Kp] so head-pair p's scores can start as
                # soon as its rope lands; V spread between pairs
                return (qp[0] + [vu[0]] + qp[1] + [vu[1]] + qp[2] + vu[2:])

            def att_blocks(qt):
                """Attention + O-projection for q block qt as emit-units,
                pipelined so head h's scores precede head h-1's context."""
                ctxq = [cxq.tile([128, 384], F16, name=f"cxq{i}")
                        for i in range(4)]
                etiles = [etp.tile([128, 16 * 512], F16, name="et")
                          for _ in range(HPC)]
                pcts = [None] * HPC
                rcs = [nrm.tile([128, 4], F32, name="rc") for _ in range(HPC)]

                def sp_unit(h, kb2):
                    # two full k-blocks through one 1024-col PSUM tile
                    p, off = h // 2, (h % 2) * 64
                    et = etiles[h]
                    psc = pps.tile([128, 1024], F32, name="psc")
                    for u in range(2):
                        kb = 2 * kb2 + u
                        nc.tensor.matmul(
                            psc[:, u * 512:(u + 1) * 512],
                            sb_k[p][off:off + 64, kb * 128:(kb + 1) * 128],
                            sb_q[p][off:off + 64, qt * 512:(qt + 1) * 512],
                            start=True, stop=True)
                    with nc.allow_low_precision(reason="f16 attn"):
                        nc.scalar.activation(
                            et[:, kb2 * 1024:(kb2 + 1) * 1024], psc[:],
                            mybir.ActivationFunctionType.Exp)

                def sd_unit(h, jj):
                    # two diagonal k-blocks (2jj, 2jj+1), masked after exp
                    p, off = h // 2, (h % 2) * 64
                    et = etiles[h]
                    psc = pps.tile([128, 1024], F32, name="psc")
                    for u in range(2):
                        j = 2 * jj + u
                        kb = 4 * qt + j
                        lo = j * 128
                        nc.tensor.matmul(
                            psc[:, u * 512 + lo:(u + 1) * 512],
                            sb_k[p][off:off + 64, kb * 128:(kb + 1) * 128],
                            sb_q[p][off:off + 64,
                                    qt * 512 + lo:(qt + 1) * 512],
                            start=True, stop=True)
                    with nc.allow_low_precision(reason="f16 attn"):
                        if jj == 0:
                            # one exp across both blocks; the 128 stale cols
                            # between them land in et cols no ctx ever reads
                            kb = 4 * qt
                            nc.scalar.activation(
                                et[:, kb * 512:(kb + 2) * 512], psc[:],
                                mybir.ActivationFunctionType.Exp)
                        else:
                            for u in range(2):
                                j = 2 * jj + u
                                kb = 4 * qt + j
                                lo = j * 128
                                esl = et[:, kb * 512 + lo:(kb + 1) * 512]
                                nc.scalar.activation(
                                    esl, psc[:, u * 512 + lo:(u + 1) * 512],
                                    mybir.ActivationFunctionType.Exp)
                        for u in range(2):
                            j = 2 * jj + u
                            kb = 4 * qt + j
                            lo = j * 128
                            msl = et[:, kb * 512 + lo:kb * 512 + lo + 128]
                            nc.gpsimd.tensor_mul(msl, msl, sb_mask[:])

                def ctx_unit(h, qc):
                    # flipped context: [128q, 65] accumulation, 65-col slots
                    # of a shared PSUM bank
                    if qc == 0:
                        pcts[h] = pcx.tile([128, 452], F32, name="pct")
                    pct = pcts[h]
                    et = etiles[h]
                    qg = 4 * qt + qc
                    psl = pct[:, qc * 65:(qc + 1) * 65]
                    for kb in range(qg + 1):
                        nc.tensor.matmul(
                            psl,
                            et[:, kb * 512 + qc * 128:kb * 512 + qc * 128 + 128],
                            sb_v[:, kb * 6 + h, :],
                            start=(kb == 0), stop=(kb == qg),
                            skip_group_check=True)
                    rc = rcs[h]
                    if qc == 0:
                        pass
                    nc.vector.reciprocal(rc[:, qc:qc + 1],
                                         pct[:, qc * 65 + 64:qc * 65 + 65])
                    with nc.allow_low_precision(reason="f16 ctx"):
                        nc.vector.tensor_scalar_mul(
                            out=ctxq[qc][:, h * 64:(h + 1) * 64],
                            in0=pct[:, qc * 65:qc * 65 + 64],
                            scalar1=rc[:, qc:qc + 1])

                def to_unit(qc):
                    # transpose ctx [q, d] -> [d, q] into the spare bytes of
                    # a pct-pool bank (bitcast to f16), then O projection
                    tb = qt * 4 + qc
                    ptt = pcx.tile([128, 452], F32, name="pct")
                    ptr = ptt[:, 260:452].bitcast(F16)
                    for p3 in range(PAIRS):
                        nc.tensor.transpose(
                            ptr[:, p3 * 128:(p3 + 1) * 128],
                            ctxq[qc][:, p3 * 128:(p3 + 1) * 128],
                            sb_iden[:])
                    with nc.allow_low_precision(reason="f16 ctxT"):
                        nc.vector.tensor_copy(
                            sb_ctxT3[:, :, tb * 128:(tb + 1) * 128],
                            ptr[:].rearrange("p (a c) -> p a c", c=128))
                    obuf = otp.tile([128, D], F32)
                    for half in range(2):
                        po = pps.tile([128, 1024], F32, name="psc")
                        for ci in range(3):
                            nc.tensor.matmul(
                                po[:, 0:384],
                                sb_ctxT[ci][:, tb * 128:(tb + 1) * 128],
                                sb_wo[:, ci * D + half * 384:
                                         ci * D + half * 384 + 384],
                                start=(ci == 0), stop=(ci == 2))
                        if half == 0:
                            nc.vector.tensor_copy(
                                obuf[:, 0:384], po[:, 0:384])
                        else:
                            nc.scalar.copy(
                                obuf[:, 384:768], po[:, 0:384])
                    # last store via SWDGE so the final two stores overlap
                    # instead of serializing on the DMA device at the drain
                    seng = nc.gpsimd if tb == 15 else nc.sync
                    seng.dma_start(
                        out_dram[tb * 128:(tb + 1) * 128, :], obuf[:])

                blocks = []
                for h in range(HPC):
                    su = [lambda h=h, kb2=kb2: sp_unit(h, kb2)
                          for kb2 in range(2 * qt)]
                    su += [lambda h=h, jj=jj: sd_unit(h, jj) for jj in range(2)]
                    if h == 0:
                        blocks.append(su)
                    else:
                        cu = [lambda h=h, qc=qc: ctx_unit(h - 1, qc)
                              for qc in range(4)]
                        blocks.append(interleave(su, cu))
                # ctx(5,qc+1) emitted before to(qc): the transpose's wait
                # on the norm never head-of-line-blocks the next ctx chain
                tail = [lambda: ctx_unit(HPC - 1, 0)]
                for qc in range(4):
                    if qc < 3:
                        tail.append(lambda qc=qc: ctx_unit(HPC - 1, qc + 1))
                    tail.append(lambda qc=qc: to_unit(qc))
                return blocks, tail

            # --- main emission: weave attention(qt-1) with proj(tt) -------
            # global software pipeline: att(qt) head blocks h2..h5 carry
            # proj(qt+1) woven in; att(qt)'s tail (ctx h5 + transpose/O,
            # latency-chain heavy) interleaves with att(qt+1)'s first two
            # head-score blocks so Act never starves at qt boundaries
            load_x(1)
            p0 = proj_units(0)
            blocks, tail = att_blocks(0)
            p0[0]()          # Q0
            p0[1]()          # K0
            for u in blocks[0]:
                u()
            for u in p0[2:]:
                u()
            for u in blocks[1]:
                u()
            prev_blocks, prev_tail = blocks, tail
            for qt in range(4):
                if qt < 3:
                    if qt + 2 <= 3:
                        load_x(qt + 2)
                    rest = [u for b in prev_blocks[2:] for u in b]
                    for u in interleave(proj_units(qt + 1), rest):
                        u()
                    nblocks, ntail = att_blocks(qt + 1)
                    pulled = nblocks[0] + nblocks[1]
                    for u in interleave(prev_tail, pulled):
                        u()
                    prev_blocks, prev_tail = nblocks, ntail
                else:
                    for b in prev_blocks[2:]:
                        for u in b:
                            u()
                    for u in prev_tail:
                        u()

            if with_collective:
                nc.gpsimd.collective_compute(
                    "AllReduce", mybir.AluOpType.add,
                    replica_groups=[[0, 1], [2, 3], [4, 5], [6, 7]],
                    ins=[bounce_in.opt()], outs=[bounce_out.opt()])
                nc.sync.dma_start(out[:], bounce_out[:])
    nc.compile()
    return nc


def make_in_maps(x, w_q, w_k, w_v, w_o, token_positions):
    # RoPE row layout: per 64-dim head-half, rows are 2 quadrant-pairs of
    # [16 even dims | 16 odd dims]; stream_shuffle swaps the 16-row halves
    # within each 32-row quadrant.
    r64 = np.arange(64)
    perm64 = 2 * (16 * (r64 // 32) + (r64 % 16)) + ((r64 % 32) >= 16)
    pos = np.asarray(token_positions).astype(np.float32)
    inv = THETA ** (-np.arange(32, dtype=np.float32) / 32.0)
    ang = inv[:, None] * pos[None, :]                       # [32 freqs, S]
    c32 = np.cos(ang).astype(np.float32)
    s32 = np.sin(ang).astype(np.float32)
    r128 = np.arange(128)
    fi = 16 * ((r128 // 32) % 2) + (r128 % 16)              # freq per row
    sgn = np.where((r128 % 32) < 16, 1.0, -1.0).astype(np.float32)
    cosd = c32[fi].astype(np.float16)                       # [128, S]
    sind = (s32[fi] * sgn[:, None]).astype(np.float16)
    kloc = np.arange(128)[:, None]
    qloc = np.arange(128)[None, :]
    maskd = (kloc <= qloc).astype(np.float16)
    idend = np.eye(128, dtype=np.float16)
    xn = np.asarray(x, dtype=np.float32)
    wqn = np.asarray(w_q, dtype=np.float32)
    wkn = np.asarray(w_k, dtype=np.float32)
    wvn = np.asarray(w_v, dtype=np.float32)
    won = np.asarray(w_o, dtype=np.float32)
    in_maps = []
    for c in range(N_CORES):
        b, hg = c // 2, c % 2
        heads = hg * HPC + np.arange(HPC)
        rows_eo = (heads[:, None] * 64 + perm64[None, :]).reshape(-1)
        wv_r = np.zeros((D, VW), np.float32)
        for h in range(HPC):
            g = hg * HPC + h
            wv_r[:, h * 65:h * 65 + 64] = wvn[g * 64:(g + 1) * 64, :].T
        wo_cols = (heads[:, None] * 64 + np.arange(64)[None, :]).reshape(-1)
        wo_r = np.ascontiguousarray(won[:, wo_cols].T).astype(np.float16)
        in_maps.append({
            "xT": np.ascontiguousarray(xn[b].T).astype(np.float16),
            "wqT": np.ascontiguousarray((wqn[rows_eo] * 0.125).T).astype(np.float16),
            "wkT": np.ascontiguousarray(wkn[rows_eo].T).astype(np.float16),
            "wv": wv_r.astype(np.float16),
            "wo": wo_r,
            "cos": cosd,
            "sin": sind,
            "mask": maskd,
            "iden": idend,
        })
    return in_maps


def kernel(x, w_q, w_k, w_v, w_o, token_positions):
    global _NC
    if _NC is None:
        _NC = build_nc()
    in_maps = make_in_maps(x, w_q, w_k, w_v, w_o, token_positions)
    res = run_bass_kernel_spmd(_NC, in_maps, core_ids=list(range(N_CORES)))
    return np.stack([res.results[2 * b]["out"] for b in range(B)], axis=0)


# revision 68
# speedup vs baseline: 1.0021x; 1.0021x over previous
import numpy as np
import concourse.bass as bass
import concourse.mybir as mybir
import concourse.tile as tile
from concourse import bacc
from concourse.bass_utils import run_bass_kernel_spmd

B, S, D, H, DH = 4, 2048, 768, 12, 64
HPC = 6          # heads per core
PAIRS = HPC // 2
THETA = 10000.0
N_CORES = 8
F32 = mybir.dt.float32
F32R = mybir.dt.float32r
F16 = mybir.dt.float16
VW = HPC * 65    # V block width: 6 heads x (64 + ones col)

# stream_shuffle swaps the 16-row even/odd halves within each 32-partition
# quadrant (RoPE rotate-half, see make_in_maps for the row layout)
SHUF_MASK = [(i + 16) % 32 for i in range(32)]

_NC = None


def interleave(main_units, extra_units):
    """Merge two unit lists, spreading extra_units evenly among main_units."""
    n, m = len(main_units), len(extra_units)
    if n == 0:
        return list(extra_units)
    res, j = [], 0
    for i, u in enumerate(main_units):
        res.append(u)
        while j < m and (j + 1) * n <= (i + 1) * m:
            res.append(extra_units[j])
            j += 1
    res.extend(extra_units[j:])
    return res


def build_nc(with_collective=True):
    nc = bacc.Bacc("TRN2", target_bir_lowering=False, debug=False,
                   num_devices=N_CORES)
    xT = nc.dram_tensor("xT", [D, S], F16, kind="ExternalInput")
    wqT = nc.dram_tensor("wqT", [D, 384], F16, kind="ExternalInput")
    wkT = nc.dram_tensor("wkT", [D, 384], F16, kind="ExternalInput")
    wv = nc.dram_tensor("wv", [D, VW], F16, kind="ExternalInput")
    wo = nc.dram_tensor("wo", [384, D], F16, kind="ExternalInput")
    cosd = nc.dram_tensor("cos", [128, S], F16, kind="ExternalInput")
    sind = nc.dram_tensor("sin", [128, S], F16, kind="ExternalInput")
    maskd = nc.dram_tensor("mask", [128, 128], F16, kind="ExternalInput")
    idend = nc.dram_tensor("iden", [128, 128], F16, kind="ExternalInput")
    out = nc.dram_tensor("out", [S, D], F32, kind="ExternalOutput")

    with tile.TileContext(nc) as tc:
        with tc.tile_pool(name="persist", bufs=1) as pp, \
             tc.tile_pool(name="dram", bufs=1, space="DRAM") as dpool, \
             tc.tile_pool(name="xp", bufs=2) as xp, \
             tc.tile_pool(name="ru", bufs=6) as ru, \
             tc.tile_pool(name="et", bufs=4) as etp, \
             tc.tile_pool(name="nrm", bufs=6) as nrm, \
             tc.tile_pool(name="cxq", bufs=3) as cxq, \
             tc.tile_pool(name="ot", bufs=3) as otp, \
             tc.tile_pool(name="pps", bufs=3, space="PSUM") as pps, \
             tc.tile_pool(name="pcx", bufs=2, space="PSUM") as pcx:
            sb_q = [pp.tile([128, S], F16, name=f"sb_q{i}") for i in range(PAIRS)]
            sb_k = [pp.tile([128, S], F16, name=f"sb_k{i}") for i in range(PAIRS)]
            sb_v = pp.tile([128, 96, 65], F16)
            sb_ctxT3 = pp.tile([128, PAIRS, S], F16)
            sb_ctxT = [sb_ctxT3[:, i, :] for i in range(PAIRS)]
            sb_wo = pp.tile([128, 3 * D], F16)
            sb_mask = pp.tile([128, 128], F16)
            sb_iden = pp.tile([128, 128], F16)
            sb_wq = pp.tile([128, 6 * 384], F16)
            sb_wk = pp.tile([128, 6 * 384], F16)
            sb_wv = pp.tile([128, 6 * VW], F16)
            sb_cos = pp.tile([128, S], F16)
            sb_sin = pp.tile([128, S], F16)
            bounce_in = dpool.tile([S, D], F32)
            bounce_out = dpool.tile([S, D], F32)
            out_dram = bounce_in if with_collective else out

            xcols = [None] * 4

            def load_x(tt):
                xcols[tt] = xp.tile([128, 6 * 512], F16, name="xcol")
                for ck in range(6):
                    nc.sync.dma_start(
                        xcols[tt][:, ck * 512:(ck + 1) * 512],
                        xT[ck * 128:(ck + 1) * 128,
                           tt * 512:(tt + 1) * 512])

            def load_mat(eng, dst, src, w):
                # one strided DMA: [nchunk*128, w] DRAM -> [128, nchunk, w]
                eng.dma_start(
                    dst[:].rearrange("p (a c) -> p a c", c=w),
                    src[:, 0:w].rearrange("(a p) c -> p a c", p=128))

            # startup: x0 per-chunk on alternating SP/Act HWDGE queues,
            # weights on SWDGE, in first-use order (x0+wq -> Q0, wk -> K0,
            # cos/sin -> rope, wv -> V)
            def load_wpair(dst, src, p):
                # one strided DMA for head-pair p's 128 weight columns
                nc.sync.dma_start(
                    dst[:].rearrange("r (a c) -> r a c", c=384)[:, :,
                                                              p * 128:
                                                              (p + 1) * 128],
                    src[:, p * 128:(p + 1) * 128].rearrange(
                        "(a r) c -> r a c", r=128))

            # startup feed order = first-use order: pair-0 Q/K weights and
            # cos/sin arrive before the later pairs
            xcols[0] = xp.tile([128, 6 * 512], F16, name="xcol")

            def load_x0(ck):
                nc.sync.dma_start(
                    xcols[0][:, ck * 512:(ck + 1) * 512],
                    xT[ck * 128:(ck + 1) * 128, 0:512])

            load_x0(0)
            load_x0(1)
            load_wpair(sb_wq, wqT, 0)
            load_wpair(sb_wk, wkT, 0)
            load_x0(2)
            load_x0(3)
            nc.sync.dma_start(sb_cos[:], cosd[:])
            nc.sync.dma_start(sb_sin[:], sind[:])
            load_x0(4)
            load_x0(5)
            load_wpair(sb_wq, wqT, 1)
            load_wpair(sb_wk, wkT, 1)
            nc.sync.dma_start(sb_mask[:], maskd[:])
            for ck in range(6):
                nc.sync.dma_start(sb_wv[:, ck * VW:(ck + 1) * VW],
                                  wv[ck * 128:(ck + 1) * 128, :])
            load_wpair(sb_wq, wqT, 2)
            load_wpair(sb_wk, wkT, 2)
            nc.sync.dma_start(sb_iden[:], idend[:])
            for ci in range(3):
                nc.sync.dma_start(sb_wo[:, ci * D:(ci + 1) * D],
                                  wo[ci * 128:(ci + 1) * 128, :])

            def proj_units(tt):
                """QKV projection + RoPE for token block tt as emit-units."""
                xcol = xcols[tt]
                csl = sb_cos[:, tt * 512:(tt + 1) * 512]
                ssl = sb_sin[:, tt * 512:(tt + 1) * 512]

                def qk_unit(wc, dst, p, wi):
                    pqt = pps.tile([128, 1024], F32, name="psc")
                    pc = pqt[:, 0:512]
                    for ck in range(6):
                        nc.tensor.matmul(
                            pc[:],
                            wc[:, ck * 384 + p * 128:ck * 384 + (p + 1) * 128],
                            xcol[:, ck * 512:(ck + 1) * 512],
                            start=(ck == 0), stop=(ck == 5))
                    dsl = dst[p][:, tt * 512:(tt + 1) * 512]
                    uh = ru.tile([128, 512], F16)
                    uhs = ru.tile([128, 512], F16)
                    with nc.allow_low_precision(reason="f16 q/k"):
                        nc.vector.tensor_mul(dsl, pc[:], csl)
                        nc.vector.tensor_mul(uh[:], pc[:], ssl)
                        nc.vector.stream_shuffle(uhs[:], uh[:], SHUF_MASK)
                        nc.vector.tensor_add(dsl, dsl, uhs[:])

                def v_unit(tj):
                    tb = tt * 4 + tj
                    pvq = pps.tile([128, 1024], F32, name="psc")
                    pvt = pvq[:, 0:512]
                    for ck in range(6):
                        nc.tensor.matmul(
                            pvt[:, 0:VW],
                            xcol[:, ck * 512 + tj * 128:
                                    ck * 512 + tj * 128 + 128],
                            sb_wv[:, ck * VW:(ck + 1) * VW],
                            start=(ck == 0), stop=(ck == 5))
                    with nc.allow_low_precision(reason="f16 V store"):
                        # late-block V copies on DVE: Act saturates with exp
                        # work in the final regions
                        dst_v = sb_v[:, tb * 6:(tb + 1) * 6, :]
                        src_v = pvt[:, 0:VW].rearrange("p (a b) -> p a b", b=65)
                        if tt < 3:
                            nc.scalar.copy(dst_v, src_v)
                        else:
                            nc.vector.tensor_copy(dst_v, src_v)
                    nc.gpsimd.memset(sb_v[:, tb * 6:(tb + 1) * 6, 64:65], 1.0)

                vu = [lambda tj=tj: v_unit(tj) for tj in range(4)]
                qp = [[lambda p=p, wi=wi, wc=wc, dst=dst: qk_unit(wc, dst, p, wi)
                       for wi, (wc, dst) in enumerate(((sb_wq, sb_q),
                                                       (sb_wk, sb_k)))]
                      for p in range(PAIRS)]
                # pair-major [Qp, Kp] so head-pair p's scores can start as
                # soon as its rope lands; V spread between pairs
                return (qp[0] + [vu[0]] + qp[1] + [vu[1]] + qp[2] + vu[2:])

            def att_blocks(qt):
                """Attention + O-projection for q block qt as emit-units,
                pipelined so head h's scores precede head h-1's context."""
                ctxq = [cxq.tile([128, 384], F16, name=f"cxq{i}")
                        for i in range(4)]
                etiles = [etp.tile([128, 16 * 512], F16, name="et")
                          for _ in range(HPC)]
                pcts = [None] * HPC
                rcs = [nrm.tile([128, 4], F32, name="rc") for _ in range(HPC)]

                def sp_unit(h, kb2):
                    # two full k-blocks through one 1024-col PSUM tile
                    p, off = h // 2, (h % 2) * 64
                    et = etiles[h]
                    psc = pps.tile([128, 1024], F32, name="psc")
                    for u in range(2):
                        kb = 2 * kb2 + u
                        nc.tensor.matmul(
                            psc[:, u * 512:(u + 1) * 512],
                            sb_k[p][off:off + 64, kb * 128:(kb + 1) * 128],
                            sb_q[p][off:off + 64, qt * 512:(qt + 1) * 512],
                            start=True, stop=True)
                    with nc.allow_low_precision(reason="f16 attn"):
                        nc.scalar.activation(
                            et[:, kb2 * 1024:(kb2 + 1) * 1024], psc[:],
                            mybir.ActivationFunctionType.Exp)

                def sd_unit(h, jj):
                    # two diagonal k-blocks (2jj, 2jj+1), masked after exp
                    p, off = h // 2, (h % 2) * 64
                    et = etiles[h]
                    psc = pps.tile([128, 1024], F32, name="psc")
                    for u in range(2):
                        j = 2 * jj + u
                        kb = 4 * qt + j
                        lo = j * 128
                        nc.tensor.matmul(
                            psc[:, u * 512 + lo:(u + 1) * 512],
                            sb_k[p][off:off + 64, kb * 128:(kb + 1) * 128],
                            sb_q[p][off:off + 64,
                                    qt * 512 + lo:(qt + 1) * 512],
                            start=True, stop=True)
                    with nc.allow_low_precision(reason="f16 attn"):
                        if jj == 0:
                            # one exp across both blocks; the 128 stale cols
                            # between them land in et cols no ctx ever reads
                            kb = 4 * qt
                            nc.scalar.activation(
                                et[:, kb * 512:(kb + 2) * 512], psc[:],
                                mybir.ActivationFunctionType.Exp)
                        else:
                            for u in range(2):
                                j = 2 * jj + u
                                kb = 4 * qt + j
                                lo = j * 128
                                esl = et[:, kb * 512 + lo:(kb + 1) * 512]
                                nc.scalar.activation(
                                    esl, psc[:, u * 512 + lo:(u + 1) * 512],
                                    mybir.ActivationFunctionType.Exp)
                        for u in range(2):
                            j = 2 * jj + u
                            kb = 4 * qt + j
                            lo = j * 128
                            msl = et[:, kb * 512 + lo:kb * 512 + lo + 128]
                            nc.gpsimd.tensor_mul(msl, msl, sb_mask[:])

                def ctx_unit(h, qc):
                    # flipped context: [128q, 65] accumulation, 65-col slots
                    # of a shared PSUM bank
                    if qc == 0:
                        pcts[h] = pcx.tile([128, 452], F32, name="pct")
                    pct = pcts[h]
                    et = etiles[h]
                    qg = 4 * qt + qc
                    psl = pct[:, qc * 65:(qc + 1) * 65]
                    for kb in range(qg + 1):
                        nc.tensor.matmul(
                            psl,
                            et[:, kb * 512 + qc * 128:kb * 512 + qc * 128 + 128],
                            sb_v[:, kb * 6 + h, :],
                            start=(kb == 0), stop=(kb == qg),
                            skip_group_check=True)
                    rc = rcs[h]
                    if qc == 0:
                        pass
                    nc.vector.reciprocal(rc[:, qc:qc + 1],
                                         pct[:, qc * 65 + 64:qc * 65 + 65])
                    with nc.allow_low_precision(reason="f16 ctx"):
                        nc.vector.tensor_scalar_mul(
                            out=ctxq[qc][:, h * 64:(h + 1) * 64],
                            in0=pct[:, qc * 65:qc * 65 + 64],
                            scalar1=rc[:, qc:qc + 1])

                def to_unit(qc):
                    # transpose ctx [q, d] -> [d, q] into the spare bytes of
                    # a pct-pool bank (bitcast to f16), then O projection
                    tb = qt * 4 + qc
                    ptt = pcx.tile([128, 452], F32, name="pct")
                    ptr = ptt[:, 260:452].bitcast(F16)
                    for p3 in range(PAIRS):
                        nc.tensor.transpose(
                            ptr[:, p3 * 128:(p3 + 1) * 128],
                            ctxq[qc][:, p3 * 128:(p3 + 1) * 128],
                            sb_iden[:])
                    with nc.allow_low_precision(reason="f16 ctxT"):
                        nc.vector.tensor_copy(
                            sb_ctxT3[:, :, tb * 128:(tb + 1) * 128],
                            ptr[:].rearrange("p (a c) -> p a c", c=128))
                    obuf = otp.tile([128, D], F32)
                    for half in range(2):
                        po = pps.tile([128, 1024], F32, name="psc")
                        for ci in range(3):
                            nc.tensor.matmul(
                                po[:, 0:384],
                                sb_ctxT[ci][:, tb * 128:(tb + 1) * 128],
                                sb_wo[:, ci * D + half * 384:
                                         ci * D + half * 384 + 384],
                                start=(ci == 0), stop=(ci == 2))
                        if half == 0:
                            nc.vector.tensor_copy(
                                obuf[:, 0:384], po[:, 0:384])
                        else:
                            nc.scalar.copy(
                                obuf[:, 384:768], po[:, 0:384])
                    nc.sync.dma_start(
                        out_dram[tb * 128:(tb + 1) * 128, :], obuf[:])

                blocks = []
                for h in range(HPC):
                    su = [lambda h=h, kb2=kb2: sp_unit(h, kb2)
                          for kb2 in range(2 * qt)]
                    su += [lambda h=h, jj=jj: sd_unit(h, jj) for jj in range(2)]
                    if h == 0:
                        blocks.append(su)
                    else:
                        cu = [lambda h=h, qc=qc: ctx_unit(h - 1, qc)
                              for qc in range(4)]
                        blocks.append(interleave(su, cu))
                # ctx(5,qc+1) emitted before to(qc): the transpose's wait
                # on the norm never head-of-line-blocks the next ctx chain
                tail = [lambda: ctx_unit(HPC - 1, 0)]
                for qc in range(4):
                    if qc < 3:
                        tail.append(lambda qc=qc: ctx_unit(HPC - 1, qc + 1))
                    tail.append(lambda qc=qc: to_unit(qc))
                return blocks, tail

            # --- main emission: weave attention(qt-1) with proj(tt) -------
            # global software pipeline: att(qt) head blocks h2..h5 carry
            # proj(qt+1) woven in; att(qt)'s tail (ctx h5 + transpose/O,
            # latency-chain heavy) interleaves with att(qt+1)'s first two
            # head-score blocks so Act never starves at qt boundaries
            load_x(1)
            p0 = proj_units(0)
            blocks, tail = att_blocks(0)
            p0[0]()          # Q0
            p0[1]()          # K0
            for u in blocks[0]:
                u()
            for u in p0[2:]:
                u()
            for u in blocks[1]:
                u()
            prev_blocks, prev_tail = blocks, tail
            for qt in range(4):
                if qt < 3:
                    if qt + 2 <= 3:
                        load_x(qt + 2)
                    rest = [u for b in prev_blocks[2:] for u in b]
                    for u in interleave(proj_units(qt + 1), rest):
                        u()
                    nblocks, ntail = att_blocks(qt + 1)
                    pulled = nblocks[0] + nblocks[1]
                    for u in interleave(prev_tail, pulled):
                        u()
                    prev_blocks, prev_tail = nblocks, ntail
                else:
                    for b in prev_blocks[2:]:
                        for u in b:
                            u()
                    for u in prev_tail:
                        u()

            if with_collective:
                nc.gpsimd.collective_compute(
                    "AllReduce", mybir.AluOpType.add,
                    replica_groups=[[0, 1], [2, 3], [4, 5], [6, 7]],
                    ins=[bounce_in.opt()], outs=[bounce_out.opt()])
                nc.sync.dma_start(out[:], bounce_out[:])
    nc.compile()
    return nc


def make_in_maps(x, w_q, w_k, w_v, w_o, token_positions):
    # RoPE row layout: per 64-dim head-half, rows are 2 quadrant-pairs of
    # [16 even dims | 16 odd dims]; stream_shuffle swaps the 16-row halves
    # within each 32-row quadrant.
    r64 = np.arange(64)
    perm64 = 2 * (16 * (r64 // 32) + (r64 % 16)) + ((r64 % 32) >= 16)
    pos = np.asarray(token_positions).astype(np.float32)
    inv = THETA ** (-np.arange(32, dtype=np.float32) / 32.0)
    ang = inv[:, None] * pos[None, :]                       # [32 freqs, S]
    c32 = np.cos(ang).astype(np.float32)
    s32 = np.sin(ang).astype(np.float32)
    r128 = np.arange(128)
    fi = 16 * ((r128 // 32) % 2) + (r128 % 16)              # freq per row
    sgn = np.where((r128 % 32) < 16, 1.0, -1.0).astype(np.float32)
    cosd = c32[fi].astype(np.float16)                       # [128, S]
    sind = (s32[fi] * sgn[:, None]).astype(np.float16)
    kloc = np.arange(128)[:, None]
    qloc = np.arange(128)[None, :]
    maskd = (kloc <= qloc).astype(np.float16)
    idend = np.eye(128, dtype=np.float16)
    xn = np.asarray(x, dtype=np.float32)
    wqn = np.asarray(w_q, dtype=np.float32)
    wkn = np.asarray(w_k, dtype=np.float32)
    wvn = np.asarray(w_v, dtype=np.float32)
    won = np.asarray(w_o, dtype=np.float32)
    in_maps = []
    for c in range(N_CORES):
        b, hg = c // 2, c % 2
        heads = hg * HPC + np.arange(HPC)
        rows_eo = (heads[:, None] * 64 + perm64[None, :]).reshape(-1)
        wv_r = np.zeros((D, VW), np.float32)
        for h in range(HPC):
            g = hg * HPC + h
            wv_r[:, h * 65:h * 65 + 64] = wvn[g * 64:(g + 1) * 64, :].T
        wo_cols = (heads[:, None] * 64 + np.arange(64)[None, :]).reshape(-1)
        wo_r = np.ascontiguousarray(won[:, wo_cols].T).astype(np.float16)
        in_maps.append({
            "xT": np.ascontiguousarray(xn[b].T).astype(np.float16),
            "wqT": np.ascontiguousarray((wqn[rows_eo] * 0.125).T).astype(np.float16),
            "wkT": np.ascontiguousarray(wkn[rows_eo].T).astype(np.float16),
            "wv": wv_r.astype(np.float16),
            "wo": wo_r,
            "cos": cosd,
            "sin": sind,
            "mask": maskd,
            "iden": idend,
        })
    return in_maps


def kernel(x, w_q, w_k, w_v, w_o, token_positions):
    global _NC
    if _NC is None:
        _NC = build_nc()
    in_maps = make_in_maps(x, w_q, w_k, w_v, w_o, token_positions)
    res = run_bass_kernel_spmd(_NC, in_maps, core_ids=list(range(N_CORES)))
    return np.stack([res.results[2 * b]["out"] for b in range(B)], axis=0)
